# revision 17
# baseline (speedup 1.0000x reference)
"""Trainium2 Bass kernel for nn_DimCosSoftmaxModule (8-core SPMD).

Math (exact refactor of the reference):
  k1[n,j,t] = relu(sum_i mem_feat[n,i] wt[i,j,t] + bt[j])                 [200,2048,3]
  k2[n,o,s] = relu(sum_{i,dt} wc[o,i,dt] k1pad[n,i,s+dt-1] + bc[o])      [200,2048,3]
  conv/sp_down fold: cls[b,n] = sum_{i,t} G[b,i,t] k2[n,i,t] + b_sp
      where G[b,i,t] = sum_u feat[b,i,u] V[u,t],  V = shifted copies of w_sp
  out = 30*(cosine(cls, w_cls) - 0.5*onehot(label))

Sharding: tensor-parallel over the 2048 channel dim (256 ch/core).
  step1 column-sharded -> AllGather k1 -> step2 o-sharded -> partial cls
  -> ReduceScatter (core c keeps batch rows 8c..8c+8) -> row-local CosFace.
Host reassembles the 8 row-shards of y (pure concatenation).

All DRAM inputs are pre-laid-out on the host so that every DMA moves
contiguous per-partition runs (>=2KB) -- descriptor-count, not bytes,
dominated the old load phase.
"""
import numpy as np
import ml_dtypes

import concourse.bass as bass
import concourse.bacc as bacc
import concourse.mybir as mybir
import concourse.tile as tile
from concourse import bass_utils
from concourse.masks import make_identity

N_CORES = 8
BS, C, HW = 64, 2048, 196
NM = 200                 # N_MEM == NUM_CLASSES
SH = C // N_CORES        # 256 channels per core
NIT = C // 128           # 16 i-tiles of 128
S_SCALE, M_MARGIN = 30.0, 0.5
RB = BS // N_CORES       # rows per core after reduce-scatter

BF16 = mybir.dt.bfloat16
F32 = mybir.dt.float32
AF = mybir.ActivationFunctionType
ALU = mybir.AluOpType

TRACE = False
TRACE_KW = {}
LAST_RESULT = None
_CACHE = {}


def build_nc():
    nc = bacc.Bacc("TRN2", target_bir_lowering=False, debug=False, num_devices=N_CORES)

    # per-core external inputs -- all big tensors pre-transposed on host so
    # each DMA is [128 partitions x contiguous run].
    MFT = nc.dram_tensor("mft", [128, NIT, NM], BF16, kind="ExternalInput")
    WT = nc.dram_tensor("wtc", [128, 3, 2, NIT, 128], BF16, kind="ExternalInput")
    WCT = nc.dram_tensor("wct", [128, 2, N_CORES, 3, SH], BF16, kind="ExternalInput")
    BTBC = nc.dram_tensor("btbc", [128, 2, 2], F32, kind="ExternalInput")
    FT = nc.dram_tensor("ftc", [HW, BS * SH], BF16, kind="ExternalInput")
    VM = nc.dram_tensor("vm", [128, 2, 3], BF16, kind="ExternalInput")
    WCLS = nc.dram_tensor("wclsT", [128, 2, NM], F32, kind="ExternalInput")
    BSP = nc.dram_tensor("bsp", [BS, 1], F32, kind="ExternalInput")   # b_sp/8, added pre-scatter
    LBL = nc.dram_tensor("lbl", [RB, 1], F32, kind="ExternalInput")
    IOTA = nc.dram_tensor("iota", [RB, NM], F32, kind="ExternalInput")
    Y = nc.dram_tensor("y", [RB, NM], F32, kind="ExternalOutput")

    NX = BS * SH          # 16384 G columns per core
    NQ = NX // 128        # 128 q-groups

    with tile.TileContext(nc) as tc:
        with (
            tc.tile_pool(name="sbuf", bufs=1) as sbuf,
            tc.tile_pool(name="psum", bufs=1, space="PSUM") as psum,
            tc.tile_pool(name="dram", bufs=1, space="DRAM") as dram,
        ):
            # ---------------- constants ----------------
            idn = sbuf.tile([128, 128], F32, tag="idn")
            make_identity(nc, idn[:])
            ones1 = sbuf.tile([1, RB], F32, tag="ones1")
            nc.vector.memset(ones1[:], 1.0)
            onesc = sbuf.tile([128, 1], F32, tag="onesc")
            nc.vector.memset(onesc[:], 1.0)
            warm_sb = sbuf.tile([128, 512], BF16, tag="warm")
            nc.vector.memset(warm_sb[:], 0.0)

            # ---------------- input DMAs ----------------
            # The SP HWDGE ring drains ~2-3x faster than the ACT ring on this
            # part, so it carries the big tensors. mem_feat + wt t=0,1 first
            # (they gate step 1); wt t=2 rides the slow ACT ring in parallel.
            mf_sb = sbuf.tile([128, NIT, NM], BF16, tag="mf")
            nc.sync.dma_start(mf_sb[:], MFT[:])                # 0.82MB
            wt_sb = sbuf.tile([128, 3, 2, NIT, 128], BF16, tag="wt")
            nc.sync.dma_start(wt_sb[:, 0], WT[:, 0])           # 1.05MB
            nc.sync.dma_start(wt_sb[:, 1], WT[:, 1])           # 1.05MB
            ft0_sb = sbuf.tile([128, NX], BF16, tag="ft0")
            nc.sync.dma_start(ft0_sb[:], FT[0:128, :])         # 4.2MB
            ft1_sb = sbuf.tile([68, NX], BF16, tag="ft1")
            nc.sync.dma_start(ft1_sb[:], FT[128:HW, :])        # 2.2MB

            nc.scalar.dma_start(wt_sb[:, 2], WT[:, 2])         # 1.05MB on ACT
            btbc_sb = sbuf.tile([128, 2, 2], F32, tag="btbc")
            nc.scalar.dma_start(btbc_sb[:], BTBC[:])
            v_sb = sbuf.tile([128, 2, 3], BF16, tag="v")
            nc.scalar.dma_start(v_sb[:], VM[:])
            wcT_sb = sbuf.tile([128, 2, N_CORES, 3, SH], BF16, tag="wcT")
            nc.scalar.dma_start(wcT_sb[:, 0], WCT[:, 0])       # 1.55MB (even i-tiles)
            nc.scalar.dma_start(wcT_sb[:, 1], WCT[:, 1])       # 1.55MB (odd i-tiles)
            wcls_sb = sbuf.tile([128, 2, NM], F32, tag="wcls")
            nc.scalar.dma_start(wcls_sb[:], WCLS[:])
            iota_sb = sbuf.tile([RB, NM], F32, tag="iota")
            nc.scalar.dma_start(iota_sb[:], IOTA[:])
            lbl_sb = sbuf.tile([RB, 1], F32, tag="lbl")
            nc.scalar.dma_start(lbl_sb[:], LBL[:])
            bsp_sb = sbuf.tile([BS, 1], F32, tag="bsp")
            nc.scalar.dma_start(bsp_sb[:], BSP[:])

            # ---------------- PE warm-up ----------------
            # ~10us of dummy matmuls while the DMAs stream, so the HAM clock
            # gate is released (1.2 -> 2.4 GHz) before step 1 begins.
            warm_ps = psum.tile([128, 512], F32, tag="ps2A", bufs=2, name="warm_ps")
            for w in range(12):
                nc.tensor.matmul(warm_ps[:], warm_sb[:, 0:128], warm_sb[:],
                                 start=True, stop=True)

            # ---------------- step 1: k1T_c[j, n] per t ----------------
            k1_sb = sbuf.tile([128, 2, 3, NM], BF16, tag="k1")
            for jc in range(2):
                for t in (0, 2, 1):
                    ps1 = psum.tile([128, NM], F32, tag="ps1", bufs=2, name=f"ps1_{t}_{jc}")
                    for it in range(NIT):
                        nc.tensor.matmul(
                            ps1[:],
                            wt_sb[:, t, jc, it, :],
                            mf_sb[:, it, :],
                            start=(it == 0), stop=(it == NIT - 1),
                        )
                    nc.vector.tensor_scalar(k1_sb[:, jc, t, :], ps1[:],
                                            btbc_sb[:, jc, 0:1], 0.0,
                                            ALU.add, ALU.max)

            # ---------------- AllGather k1 (two pipelined halves) ----------------
            # Half jc is bounced+gathered as soon as its 3 relus land, so
            # step-2's h=0 tiles start while half 1 is still on the ring.
            kbs, kgs, k1fs = [], [], []
            for jc in range(2):
                kb = dram.tile([128, 3, NM], BF16, name=f"k1_bounce{jc}")
                kg = dram.tile([N_CORES, 128, 3, NM], BF16, name=f"k1_gath{jc}",
                               addr_space="Shared")
                nc.gpsimd.dma_start(kb[:], k1_sb[:, jc])
                nc.gpsimd.collective_compute(
                    "AllGather", ALU.bypass,
                    replica_groups=[list(range(N_CORES))],
                    ins=[kb.opt()], outs=[kg.opt()],
                )
                kbs.append(kb); kgs.append(kg)
            for jc in range(2):
                k1fh = sbuf.tile([128, N_CORES, 3, NM], BF16, tag=f"k1f{jc}",
                                 name=f"k1f{jc}")
                nc.sync.dma_start(k1fh[:], kgs[jc].rearrange("g p t n -> p g t n"))
                k1fs.append(k1fh)

            # ---------------- G: featT-stationary matmuls ----------------
            # out[x-chunk, t] = sum_u featT[u, x] V[u, t]; 64 chunks packed per
            # PSUM bank, one DVE cast-copy per bank into gbuf16.
            # gbuf16 free index = 3*q + t with q = chunk = b*2 + h.
            gbuf16 = sbuf.tile([128, NQ * 3], BF16, tag="gbuf16")
            CPB = 64                       # chunks per bank
            nbanks = (NQ + CPB - 1) // CPB
            for bank in range(nbanks):
                c0 = bank * CPB
                c1 = min(c0 + CPB, NQ)
                gpk = psum.tile([128, CPB * 3], F32, tag="gpk", bufs=2, name=f"gpk{bank}")
                for c in range(c0, c1):
                    col = (c - c0) * 3
                    nc.tensor.matmul(gpk[:, col:col + 3],
                                     ft0_sb[:, c * 128:(c + 1) * 128],
                                     v_sb[0:128, 0, :], start=True, stop=False)
                    nc.tensor.matmul(gpk[:, col:col + 3],
                                     ft1_sb[0:68, c * 128:(c + 1) * 128],
                                     v_sb[0:68, 1, :], start=False, stop=True)
                nc.vector.tensor_copy(gbuf16[:, c0 * 3:c1 * 3], gpk[:, 0:(c1 - c0) * 3])

            # ---------------- step 2: k2T_s[o, n] ----------------
            k2_sb = sbuf.tile([128, 2, 3, NM], BF16, tag="k2")
            for oc in range(2):
                # bank A holds s=0,1 (N=400), bank B holds s=2 (N=200)
                psA = psum.tile([128, 2 * NM], F32, tag="ps2A", bufs=2, name=f"ps2A_{oc}")
                psB = psum.tile([128, NM], F32, tag="ps2B", bufs=1, name=f"ps2B_{oc}")
                n_it = 0
                for h in range(2):
                    for g in range(N_CORES):
                        it = 2 * g + h      # global 128-j tile index
                        first = (n_it == 0)
                        last = (n_it == 2 * N_CORES - 1)
                        kv = k1fs[h][:, g].rearrange("p t n -> p (t n)")
                        l0 = wcT_sb[:, h, g, 0, oc * 128:(oc + 1) * 128]
                        l1 = wcT_sb[:, h, g, 1, oc * 128:(oc + 1) * 128]
                        l2 = wcT_sb[:, h, g, 2, oc * 128:(oc + 1) * 128]
                        # dt=1: t'=0,1 -> s=0,1 (A[0:400])
                        nc.tensor.matmul(psA[:, 0:2 * NM], l1, kv[:, 0:2 * NM],
                                         start=first, stop=False)
                        # dt=0: t'=0 -> s=1 (A[200:400])
                        nc.tensor.matmul(psA[:, NM:2 * NM], l0, kv[:, 0:NM],
                                         start=False, stop=False)
                        # dt=2: t'=1,2 -> s=0,1 (A[0:400])
                        nc.tensor.matmul(psA[:, 0:2 * NM], l2, kv[:, NM:3 * NM],
                                         start=False, stop=last)
                        # dt=0: t'=1 -> s=2 (B)
                        nc.tensor.matmul(psB[:], l0, kv[:, NM:2 * NM],
                                         start=first, stop=False)
                        # dt=1: t'=2 -> s=2 (B)
                        nc.tensor.matmul(psB[:], l1, kv[:, 2 * NM:3 * NM],
                                         start=False, stop=last)
                        n_it += 1
                nc.vector.tensor_scalar(k2_sb[:, oc, 0, :], psA[:, 0:NM],
                                        btbc_sb[:, oc, 1:2], 0.0, ALU.add, ALU.max)
                nc.vector.tensor_scalar(k2_sb[:, oc, 1, :], psA[:, NM:2 * NM],
                                        btbc_sb[:, oc, 1:2], 0.0, ALU.add, ALU.max)
                nc.vector.tensor_scalar(k2_sb[:, oc, 2, :], psB[:],
                                        btbc_sb[:, oc, 1:2], 0.0, ALU.add, ALU.max)

            # ---------------- cls partial: [64, 200] ----------------
            cps = psum.tile([BS, NM], F32, tag="ep", name="cps")
            first = True
            for h in range(2):
                for t in range(3):
                    lhs = gbuf16[:, 3 * h + t::6]
                    nc.tensor.matmul(cps[:], lhs[:, 0:BS], k2_sb[:, h, t, :],
                                     start=first, stop=(h == 1 and t == 2))
                    first = False
            clsp_sb = sbuf.tile([BS, NM], F32, tag="clsp")
            nc.vector.tensor_scalar(clsp_sb[:], cps[:], bsp_sb[:], None, ALU.add)

            # ---------------- ReduceScatter cls (core c keeps rows 8c..8c+8) ----------------
            cls_bounce = dram.tile([BS, NM], F32, name="cls_bounce")
            cls_red = dram.tile([RB, NM], F32, name="cls_red")
            nc.sync.dma_start(cls_bounce[:], clsp_sb[:])
            nc.gpsimd.collective_compute(
                "ReduceScatter", ALU.add,
                replica_groups=[list(range(N_CORES))],
                ins=[cls_bounce.opt()], outs=[cls_red.opt()],
            )
            cls_sb = sbuf.tile([RB, NM], F32, tag="cls")
            nc.sync.dma_start(cls_sb[:], cls_red[:])

            # ---------------- CosFace epilogue ----------------
            # Pre-compute everything that only needs w_cls / iota / label so it
            # overlaps the collectives; keep the post-ReduceScatter chain short.
            wsq_sb = sbuf.tile([128, 2, NM], F32, tag="wsq")
            nc.scalar.activation(wsq_sb[0:128, 0, :], wcls_sb[0:128, 0, :], AF.Square)
            nc.scalar.activation(wsq_sb[0:72, 1, :], wcls_sb[0:72, 1, :], AF.Square)
            wsA = psum.tile([128, 1], F32, tag="ep", name="wsA")
            nc.tensor.matmul(wsA[:], wsq_sb[0:128, 0, 0:128], onesc[0:128, :], start=True, stop=False)
            nc.tensor.matmul(wsA[:], wsq_sb[0:72, 1, 0:128], onesc[0:72, :], start=False, stop=True)
            wsB = psum.tile([72, 1], F32, tag="ep", name="wsB")
            nc.tensor.matmul(wsB[:], wsq_sb[0:128, 0, 128:NM], onesc[0:128, :], start=True, stop=False)
            nc.tensor.matmul(wsB[:], wsq_sb[0:72, 1, 128:NM], onesc[0:72, :], start=False, stop=True)
            wnorm_sb = sbuf.tile([128, 2], F32, tag="wnorm")
            nc.scalar.activation(wnorm_sb[:, 0:1], wsA[:], AF.Sqrt)
            nc.scalar.activation(wnorm_sb[0:72, 1:2], wsB[:], AF.Sqrt)
            winv_sb = sbuf.tile([128, 2], F32, tag="winv")
            nc.vector.reciprocal(winv_sb[:, 0:1], wnorm_sb[:, 0:1])
            nc.vector.reciprocal(winv_sb[0:72, 1:2], wnorm_sb[0:72, 1:2])
            winvrow_sb = sbuf.tile([1, NM], F32, tag="winvrow")
            wr1 = psum.tile([1, 128], F32, tag="ep", name="wr1")
            nc.tensor.transpose(wr1[:], winv_sb[:, 0:1], idn[:])
            nc.vector.tensor_copy(winvrow_sb[:, 0:128], wr1[:])
            wr2 = psum.tile([1, 72], F32, tag="ep", name="wr2")
            nc.tensor.transpose(wr2[:], winv_sb[0:72, 1:2], idn[0:72, 0:72])
            nc.vector.tensor_copy(winvrow_sb[:, 128:NM], wr2[:])
            wbps = psum.tile([RB, NM], F32, tag="ep", name="wbps")
            nc.tensor.matmul(wbps[:], ones1[:], winvrow_sb[:], start=True, stop=True)
            winvbS_sb = sbuf.tile([RB, NM], F32, tag="winvbS")
            nc.scalar.mul(winvbS_sb[:], wbps[:], S_SCALE)          # S/|w_c| broadcast
            maskSM_sb = sbuf.tile([RB, NM], F32, tag="maskSM")
            nc.vector.tensor_scalar(maskSM_sb[:], iota_sb[:], lbl_sb[:], None, ALU.is_equal)
            nc.vector.tensor_scalar(maskSM_sb[:], maskSM_sb[:], S_SCALE * M_MARGIN, None, ALU.mult)

            # ---- post-ReduceScatter chain (b_sp already folded in pre-scatter) ----
            sq_sb = sbuf.tile([RB, NM], F32, tag="sq")
            ss_sb = sbuf.tile([RB, 1], F32, tag="ss")
            nc.scalar.activation(sq_sb[:], cls_sb[:], AF.Square, accum_out=ss_sb[:])
            rt_sb = sbuf.tile([RB, 1], F32, tag="rt")
            nc.scalar.activation(rt_sb[:], ss_sb[:], AF.Sqrt)
            invx_sb = sbuf.tile([RB, 1], F32, tag="invx")
            nc.vector.reciprocal(invx_sb[:], rt_sb[:])
            clsT_sb = sbuf.tile([128, 2, RB], F32, tag="clsT")
            tp1 = psum.tile([128, RB], F32, tag="ep", name="tp1")
            nc.tensor.transpose(tp1[:], cls_sb[:, 0:128], idn[0:RB, 0:RB])
            nc.vector.tensor_copy(clsT_sb[0:128, 0, :], tp1[:])
            tp2 = psum.tile([72, RB], F32, tag="ep", name="tp2")
            nc.tensor.transpose(tp2[:], cls_sb[:, 128:NM], idn[0:RB, 0:RB])
            nc.vector.tensor_copy(clsT_sb[0:72, 1, :], tp2[:])
            cos_ps = psum.tile([RB, NM], F32, tag="ep", name="cos_ps")
            nc.tensor.matmul(cos_ps[:], clsT_sb[0:128, 0, :], wcls_sb[0:128, 0, :],
                             start=True, stop=False)
            nc.tensor.matmul(cos_ps[:], clsT_sb[0:72, 1, :], wcls_sb[0:72, 1, :],
                             start=False, stop=True)
            t1_sb = sbuf.tile([RB, NM], F32, tag="t1")
            nc.vector.scalar_tensor_tensor(t1_sb[:], cos_ps[:], invx_sb[:],
                                           winvbS_sb[:], ALU.mult, ALU.mult)
            out_sb = sbuf.tile([RB, NM], F32, tag="out")
            nc.vector.tensor_tensor(out_sb[:], t1_sb[:], maskSM_sb[:], ALU.subtract)
            nc.sync.dma_start(Y[:], out_sb[:])

    nc.compile()
    return nc


def _prep_inputs(feat, label, mem_feat, wt, bt, wc, bc, w_sp, b_sp, w_cls):
    bf = ml_dtypes.bfloat16
    f32 = np.float32
    feat = np.ascontiguousarray(np.asarray(feat, dtype=f32))
    mem_feat = np.asarray(mem_feat, dtype=f32)
    wt = np.asarray(wt, dtype=f32)
    bt = np.asarray(bt, dtype=f32)
    wc = np.asarray(wc, dtype=f32)
    bc = np.asarray(bc, dtype=f32)
    w_sp = np.asarray(w_sp, dtype=f32)
    b_sp = np.asarray(b_sp, dtype=f32)
    w_cls = np.asarray(w_cls, dtype=f32)
    label = np.asarray(label)

    V = np.zeros((HW, 3), f32)
    V[:HW - 1, 0] = w_sp[0, 1:]
    V[:, 1] = w_sp[0, :]
    V[1:, 2] = w_sp[0, :HW - 1]
    vm = np.zeros((128, 2, 3), f32)
    vm[:, 0, :] = V[0:128]
    vm[0:68, 1, :] = V[128:HW]
    vm = vm.astype(bf)

    # mem_feat.T [2048,200] -> [128, 16, 200]
    mft = np.ascontiguousarray(
        mem_feat.T.reshape(NIT, 128, NM).transpose(1, 0, 2)).astype(bf)

    wclsT = np.zeros((128, 2, NM), f32)
    wclsT[:, 0, :] = w_cls.T[0:128]
    wclsT[0:72, 1, :] = w_cls.T[128:NM]

    bsp = np.full((BS, 1), b_sp[0] / N_CORES, f32)
    lbl_full = label.astype(f32).reshape(BS, 1)
    iota = np.broadcast_to(np.arange(NM, dtype=f32), (RB, NM)).copy()

    fv = feat.reshape(BS, C, HW)
    in_maps = []
    for c in range(N_CORES):
        J = slice(c * SH, (c + 1) * SH)
        # wt_sb[p, t, jc, it, j'] = wt[it*128+p, c*256+jc*128+j', t]
        wt_c = np.ascontiguousarray(
            wt[:, J, :].reshape(NIT, 128, 2, 128, 3)
            .transpose(1, 4, 2, 0, 3)).astype(bf)
        # wcT_sb[p, it, dt, o'] = wc[c*256+o', it*128+p, dt]
        wct_c = np.ascontiguousarray(
            wc[J].transpose(1, 2, 0).reshape(N_CORES, 2, 128, 3, SH)
            .transpose(2, 1, 0, 3, 4)).astype(bf)
        in_maps.append({
            "mft": mft,
            "wtc": wt_c,
            "wct": wct_c,
            "btbc": np.ascontiguousarray(
                np.stack([bt[J].reshape(2, 128).T, bc[J].reshape(2, 128).T],
                         axis=-1)),
            "ftc": np.ascontiguousarray(
                fv[:, J, :].transpose(2, 0, 1).reshape(HW, BS * SH)).astype(bf),
            "vm": vm, "wclsT": wclsT,
            "bsp": bsp, "lbl": lbl_full[c * RB:(c + 1) * RB], "iota": iota,
        })
    return in_maps


def kernel(**inputs) -> np.ndarray:
    global LAST_RESULT
    if "nc" not in _CACHE:
        _CACHE["nc"] = build_nc()
    nc = _CACHE["nc"]
    in_maps = _prep_inputs(**inputs)
    try:
        res = bass_utils.run_bass_kernel_spmd(
            nc, in_maps, core_ids=list(range(N_CORES)),
            trace=TRACE, **TRACE_KW,
        )
    except Exception:
        # transient NRT/device hiccups recover on retry
        res = bass_utils.run_bass_kernel_spmd(
            nc, in_maps, core_ids=list(range(N_CORES)),
            trace=TRACE, **TRACE_KW,
        )
    LAST_RESULT = res
    return np.concatenate(
        [np.asarray(res.results[c]["y"], dtype=np.float32) for c in range(N_CORES)],
        axis=0,
    )


# revision 18
# speedup vs baseline: 1.1081x; 1.1081x over previous
"""Trainium2 Bass kernel for nn_DimCosSoftmaxModule (8-core SPMD).

Math (exact refactor of the reference):
  k1[n,j,t] = relu(sum_i mem_feat[n,i] wt[i,j,t] + bt[j])                 [200,2048,3]
  k2[n,o,s] = relu(sum_{i,dt} wc[o,i,dt] k1pad[n,i,s+dt-1] + bc[o])      [200,2048,3]
  conv/sp_down fold: cls[b,n] = sum_{i,t} G[b,i,t] k2[n,i,t] + b_sp
      where G[b,i,t] = sum_u feat[b,i,u] V[u,t],  V = shifted copies of w_sp
  out = 30*(cosine(cls, w_cls) - 0.5*onehot(label))

Sharding: tensor-parallel over the 2048 channel dim (256 ch/core).
  step1 column-sharded -> AllGather k1 -> step2 o-sharded -> partial cls
  -> ReduceScatter (core c keeps batch rows 8c..8c+8) -> row-local CosFace.
Host reassembles the 8 row-shards of y (pure concatenation).

All DRAM inputs are pre-laid-out on the host so that every DMA moves
contiguous per-partition runs (>=2KB) -- descriptor-count, not bytes,
dominated the old load phase.
"""
import numpy as np
import ml_dtypes

import concourse.bass as bass
import concourse.bacc as bacc
import concourse.mybir as mybir
import concourse.tile as tile
from concourse import bass_utils
from concourse.masks import make_identity

N_CORES = 8
BS, C, HW = 64, 2048, 196
NM = 200                 # N_MEM == NUM_CLASSES
SH = C // N_CORES        # 256 channels per core
NIT = C // 128           # 16 i-tiles of 128
S_SCALE, M_MARGIN = 30.0, 0.5
RB = BS // N_CORES       # rows per core after reduce-scatter

BF16 = mybir.dt.bfloat16
F32 = mybir.dt.float32
AF = mybir.ActivationFunctionType
ALU = mybir.AluOpType

TRACE = False
TRACE_KW = {}
LAST_RESULT = None
_CACHE = {}


def build_nc():
    nc = bacc.Bacc("TRN2", target_bir_lowering=False, debug=False, num_devices=N_CORES)

    # per-core external inputs -- all big tensors pre-transposed on host so
    # each DMA is [128 partitions x contiguous run].
    MFT = nc.dram_tensor("mft", [128, NIT, NM], BF16, kind="ExternalInput")
    WT = nc.dram_tensor("wtc", [128, 3, 2, NIT, 128], BF16, kind="ExternalInput")
    WCT = nc.dram_tensor("wct", [128, 2, N_CORES, 3, SH], BF16, kind="ExternalInput")
    BTBC = nc.dram_tensor("btbc", [128, 2, 2], F32, kind="ExternalInput")
    FT = nc.dram_tensor("ftc", [HW, BS * SH], BF16, kind="ExternalInput")
    VM = nc.dram_tensor("vm", [128, 2, 3], BF16, kind="ExternalInput")
    WCLS = nc.dram_tensor("wclsT", [128, 2, NM], F32, kind="ExternalInput")
    BSP = nc.dram_tensor("bsp", [BS, 1], F32, kind="ExternalInput")   # b_sp/8, added pre-scatter
    LBL = nc.dram_tensor("lbl", [RB, 1], F32, kind="ExternalInput")
    IOTA = nc.dram_tensor("iota", [RB, NM], F32, kind="ExternalInput")
    Y = nc.dram_tensor("y", [RB, NM], F32, kind="ExternalOutput")

    NX = BS * SH          # 16384 G columns per core
    NQ = NX // 128        # 128 q-groups

    with tile.TileContext(nc) as tc:
        with (
            tc.tile_pool(name="sbuf", bufs=1) as sbuf,
            tc.tile_pool(name="psum", bufs=1, space="PSUM") as psum,
            tc.tile_pool(name="dram", bufs=1, space="DRAM") as dram,
        ):
            # ---------------- constants ----------------
            idn = sbuf.tile([128, 128], F32, tag="idn")
            make_identity(nc, idn[:])
            ones1 = sbuf.tile([1, RB], F32, tag="ones1")
            nc.vector.memset(ones1[:], 1.0)
            onesc = sbuf.tile([128, 1], F32, tag="onesc")
            nc.vector.memset(onesc[:], 1.0)
            warm_sb = sbuf.tile([128, 512], BF16, tag="warm")
            nc.vector.memset(warm_sb[:], 0.0)

            # ---------------- input DMAs ----------------
            # The SP HWDGE ring drains ~2-3x faster than the ACT ring on this
            # part, so it carries the big tensors. mem_feat + wt t=0,1 first
            # (they gate step 1); wt t=2 rides the slow ACT ring in parallel.
            mf_sb = sbuf.tile([128, NIT, NM], BF16, tag="mf")
            nc.sync.dma_start(mf_sb[:], MFT[:])                # 0.82MB
            wt_sb = sbuf.tile([128, 3, 2, NIT, 128], BF16, tag="wt")
            nc.sync.dma_start(wt_sb[:, 0], WT[:, 0])           # 1.05MB
            nc.sync.dma_start(wt_sb[:, 1], WT[:, 1])           # 1.05MB
            ft0_sb = sbuf.tile([128, NX], BF16, tag="ft0")
            nc.sync.dma_start(ft0_sb[:], FT[0:128, :])         # 4.2MB
            ft1_sb = sbuf.tile([68, NX], BF16, tag="ft1")
            nc.sync.dma_start(ft1_sb[:], FT[128:HW, :])        # 2.2MB

            nc.scalar.dma_start(wt_sb[:, 2], WT[:, 2])         # 1.05MB on ACT
            btbc_sb = sbuf.tile([128, 2, 2], F32, tag="btbc")
            nc.scalar.dma_start(btbc_sb[:], BTBC[:])
            v_sb = sbuf.tile([128, 2, 3], BF16, tag="v")
            nc.scalar.dma_start(v_sb[:], VM[:])
            wcT_sb = sbuf.tile([128, 2, N_CORES, 3, SH], BF16, tag="wcT")
            nc.scalar.dma_start(wcT_sb[:, 0], WCT[:, 0])       # 1.55MB (even i-tiles)
            nc.scalar.dma_start(wcT_sb[:, 1], WCT[:, 1])       # 1.55MB (odd i-tiles)
            wcls_sb = sbuf.tile([128, 2, NM], F32, tag="wcls")
            nc.scalar.dma_start(wcls_sb[:], WCLS[:])
            iota_sb = sbuf.tile([RB, NM], F32, tag="iota")
            nc.scalar.dma_start(iota_sb[:], IOTA[:])
            lbl_sb = sbuf.tile([RB, 1], F32, tag="lbl")
            nc.scalar.dma_start(lbl_sb[:], LBL[:])
            bsp_sb = sbuf.tile([BS, 1], F32, tag="bsp")
            nc.scalar.dma_start(bsp_sb[:], BSP[:])

            # ---------------- PE warm-up ----------------
            # ~10us of dummy matmuls while the DMAs stream, so the HAM clock
            # gate is released (1.2 -> 2.4 GHz) before step 1 begins.
            warm_ps = psum.tile([128, 512], F32, tag="ps2A", bufs=2, name="warm_ps")
            for w in range(12):
                nc.tensor.matmul(warm_ps[:], warm_sb[:, 0:128], warm_sb[:],
                                 start=True, stop=True)

            # ---------------- step 1: k1T_c[j, n] per t ----------------
            k1_sb = sbuf.tile([128, 2, 3, NM], BF16, tag="k1")
            for jc in range(2):
                for t in (0, 2, 1):
                    ps1 = psum.tile([128, NM], F32, tag="ps1", bufs=2, name=f"ps1_{t}_{jc}")
                    for it in range(NIT):
                        nc.tensor.matmul(
                            ps1[:],
                            wt_sb[:, t, jc, it, :],
                            mf_sb[:, it, :],
                            start=(it == 0), stop=(it == NIT - 1),
                        )
                    nc.vector.tensor_scalar(k1_sb[:, jc, t, :], ps1[:],
                                            btbc_sb[:, jc, 0:1], 0.0,
                                            ALU.add, ALU.max)

            # ---------------- AllGather k1 ----------------
            kb = dram.tile([128, 2, 3, NM], BF16, name="k1_bounce")
            kg = dram.tile([N_CORES, 128, 2, 3, NM], BF16, name="k1_gath",
                           addr_space="Shared")
            nc.gpsimd.dma_start(kb[:], k1_sb[:])
            nc.gpsimd.collective_compute(
                "AllGather", ALU.bypass,
                replica_groups=[list(range(N_CORES))],
                ins=[kb.opt()], outs=[kg.opt()],
            )
            k1f = sbuf.tile([128, N_CORES, 2, 3, NM], BF16, tag="k1f")
            nc.sync.dma_start(k1f[:], kg.rearrange("g p a t n -> p g a t n"))

            # ---------------- G: featT-stationary matmuls ----------------
            # out[x-chunk, t] = sum_u featT[u, x] V[u, t]; 64 chunks packed per
            # PSUM bank, one DVE cast-copy per bank into gbuf16.
            # gbuf16 free index = 3*q + t with q = chunk = b*2 + h.
            gbuf16 = sbuf.tile([128, NQ * 3], BF16, tag="gbuf16")
            CPB = 64                       # chunks per bank
            nbanks = (NQ + CPB - 1) // CPB
            for bank in range(nbanks):
                c0 = bank * CPB
                c1 = min(c0 + CPB, NQ)
                gpk = psum.tile([128, CPB * 3], F32, tag="gpk", bufs=2, name=f"gpk{bank}")
                for c in range(c0, c1):
                    col = (c - c0) * 3
                    nc.tensor.matmul(gpk[:, col:col + 3],
                                     ft0_sb[:, c * 128:(c + 1) * 128],
                                     v_sb[0:128, 0, :], start=True, stop=False)
                    nc.tensor.matmul(gpk[:, col:col + 3],
                                     ft1_sb[0:68, c * 128:(c + 1) * 128],
                                     v_sb[0:68, 1, :], start=False, stop=True)
                nc.vector.tensor_copy(gbuf16[:, c0 * 3:c1 * 3], gpk[:, 0:(c1 - c0) * 3])

            # ---------------- step 2: k2T_s[o, n] ----------------
            k2_sb = sbuf.tile([128, 2, 3, NM], BF16, tag="k2")
            for oc in range(2):
                # bank A holds s=0,1 (N=400), bank B holds s=2 (N=200)
                psA = psum.tile([128, 2 * NM], F32, tag="ps2A", bufs=2, name=f"ps2A_{oc}")
                psB = psum.tile([128, NM], F32, tag="ps2B", bufs=1, name=f"ps2B_{oc}")
                n_it = 0
                for h in range(2):
                    for g in range(N_CORES):
                        it = 2 * g + h      # global 128-j tile index
                        first = (n_it == 0)
                        last = (n_it == 2 * N_CORES - 1)
                        kv = k1f[:, g, h].rearrange("p t n -> p (t n)")
                        l0 = wcT_sb[:, h, g, 0, oc * 128:(oc + 1) * 128]
                        l1 = wcT_sb[:, h, g, 1, oc * 128:(oc + 1) * 128]
                        l2 = wcT_sb[:, h, g, 2, oc * 128:(oc + 1) * 128]
                        # dt=1: t'=0,1 -> s=0,1 (A[0:400])
                        nc.tensor.matmul(psA[:, 0:2 * NM], l1, kv[:, 0:2 * NM],
                                         start=first, stop=False)
                        # dt=0: t'=0 -> s=1 (A[200:400])
                        nc.tensor.matmul(psA[:, NM:2 * NM], l0, kv[:, 0:NM],
                                         start=False, stop=False)
                        # dt=2: t'=1,2 -> s=0,1 (A[0:400])
                        nc.tensor.matmul(psA[:, 0:2 * NM], l2, kv[:, NM:3 * NM],
                                         start=False, stop=last)
                        # dt=0: t'=1 -> s=2 (B)
                        nc.tensor.matmul(psB[:], l0, kv[:, NM:2 * NM],
                                         start=first, stop=False)
                        # dt=1: t'=2 -> s=2 (B)
                        nc.tensor.matmul(psB[:], l1, kv[:, 2 * NM:3 * NM],
                                         start=False, stop=last)
                        n_it += 1
                nc.vector.tensor_scalar(k2_sb[:, oc, 0, :], psA[:, 0:NM],
                                        btbc_sb[:, oc, 1:2], 0.0, ALU.add, ALU.max)
                nc.vector.tensor_scalar(k2_sb[:, oc, 1, :], psA[:, NM:2 * NM],
                                        btbc_sb[:, oc, 1:2], 0.0, ALU.add, ALU.max)
                nc.vector.tensor_scalar(k2_sb[:, oc, 2, :], psB[:],
                                        btbc_sb[:, oc, 1:2], 0.0, ALU.add, ALU.max)

            # ---------------- cls partial: [64, 200] ----------------
            cps = psum.tile([BS, NM], F32, tag="ep", name="cps")
            first = True
            for h in range(2):
                for t in range(3):
                    lhs = gbuf16[:, 3 * h + t::6]
                    nc.tensor.matmul(cps[:], lhs[:, 0:BS], k2_sb[:, h, t, :],
                                     start=first, stop=(h == 1 and t == 2))
                    first = False
            clsp_sb = sbuf.tile([BS, NM], F32, tag="clsp")
            nc.vector.tensor_scalar(clsp_sb[:], cps[:], bsp_sb[:], None, ALU.add)

            # ---------------- ReduceScatter cls (core c keeps rows 8c..8c+8) ----------------
            cls_bounce = dram.tile([BS, NM], F32, name="cls_bounce")
            cls_red = dram.tile([RB, NM], F32, name="cls_red")
            nc.sync.dma_start(cls_bounce[:], clsp_sb[:])
            nc.gpsimd.collective_compute(
                "ReduceScatter", ALU.add,
                replica_groups=[list(range(N_CORES))],
                ins=[cls_bounce.opt()], outs=[cls_red.opt()],
            )
            cls_sb = sbuf.tile([RB, NM], F32, tag="cls")
            nc.sync.dma_start(cls_sb[:], cls_red[:])

            # ---------------- CosFace epilogue ----------------
            # Pre-compute everything that only needs w_cls / iota / label so it
            # overlaps the collectives; keep the post-ReduceScatter chain short.
            wsq_sb = sbuf.tile([128, 2, NM], F32, tag="wsq")
            nc.scalar.activation(wsq_sb[0:128, 0, :], wcls_sb[0:128, 0, :], AF.Square)
            nc.scalar.activation(wsq_sb[0:72, 1, :], wcls_sb[0:72, 1, :], AF.Square)
            wsA = psum.tile([128, 1], F32, tag="ep", name="wsA")
            nc.tensor.matmul(wsA[:], wsq_sb[0:128, 0, 0:128], onesc[0:128, :], start=True, stop=False)
            nc.tensor.matmul(wsA[:], wsq_sb[0:72, 1, 0:128], onesc[0:72, :], start=False, stop=True)
            wsB = psum.tile([72, 1], F32, tag="ep", name="wsB")
            nc.tensor.matmul(wsB[:], wsq_sb[0:128, 0, 128:NM], onesc[0:128, :], start=True, stop=False)
            nc.tensor.matmul(wsB[:], wsq_sb[0:72, 1, 128:NM], onesc[0:72, :], start=False, stop=True)
            wnorm_sb = sbuf.tile([128, 2], F32, tag="wnorm")
            nc.scalar.activation(wnorm_sb[:, 0:1], wsA[:], AF.Sqrt)
            nc.scalar.activation(wnorm_sb[0:72, 1:2], wsB[:], AF.Sqrt)
            winv_sb = sbuf.tile([128, 2], F32, tag="winv")
            nc.vector.reciprocal(winv_sb[:, 0:1], wnorm_sb[:, 0:1])
            nc.vector.reciprocal(winv_sb[0:72, 1:2], wnorm_sb[0:72, 1:2])
            winvrow_sb = sbuf.tile([1, NM], F32, tag="winvrow")
            wr1 = psum.tile([1, 128], F32, tag="ep", name="wr1")
            nc.tensor.transpose(wr1[:], winv_sb[:, 0:1], idn[:])
            nc.vector.tensor_copy(winvrow_sb[:, 0:128], wr1[:])
            wr2 = psum.tile([1, 72], F32, tag="ep", name="wr2")
            nc.tensor.transpose(wr2[:], winv_sb[0:72, 1:2], idn[0:72, 0:72])
            nc.vector.tensor_copy(winvrow_sb[:, 128:NM], wr2[:])
            wbps = psum.tile([RB, NM], F32, tag="ep", name="wbps")
            nc.tensor.matmul(wbps[:], ones1[:], winvrow_sb[:], start=True, stop=True)
            winvbS_sb = sbuf.tile([RB, NM], F32, tag="winvbS")
            nc.scalar.mul(winvbS_sb[:], wbps[:], S_SCALE)          # S/|w_c| broadcast
            maskSM_sb = sbuf.tile([RB, NM], F32, tag="maskSM")
            nc.vector.tensor_scalar(maskSM_sb[:], iota_sb[:], lbl_sb[:], None, ALU.is_equal)
            nc.vector.tensor_scalar(maskSM_sb[:], maskSM_sb[:], S_SCALE * M_MARGIN, None, ALU.mult)

            # ---- post-ReduceScatter chain (b_sp already folded in pre-scatter) ----
            sq_sb = sbuf.tile([RB, NM], F32, tag="sq")
            ss_sb = sbuf.tile([RB, 1], F32, tag="ss")
            nc.scalar.activation(sq_sb[:], cls_sb[:], AF.Square, accum_out=ss_sb[:])
            rt_sb = sbuf.tile([RB, 1], F32, tag="rt")
            nc.scalar.activation(rt_sb[:], ss_sb[:], AF.Sqrt)
            invx_sb = sbuf.tile([RB, 1], F32, tag="invx")
            nc.vector.reciprocal(invx_sb[:], rt_sb[:])
            clsT_sb = sbuf.tile([128, 2, RB], F32, tag="clsT")
            tp1 = psum.tile([128, RB], F32, tag="ep", name="tp1")
            nc.tensor.transpose(tp1[:], cls_sb[:, 0:128], idn[0:RB, 0:RB])
            nc.vector.tensor_copy(clsT_sb[0:128, 0, :], tp1[:])
            tp2 = psum.tile([72, RB], F32, tag="ep", name="tp2")
            nc.tensor.transpose(tp2[:], cls_sb[:, 128:NM], idn[0:RB, 0:RB])
            nc.vector.tensor_copy(clsT_sb[0:72, 1, :], tp2[:])
            cos_ps = psum.tile([RB, NM], F32, tag="ep", name="cos_ps")
            nc.tensor.matmul(cos_ps[:], clsT_sb[0:128, 0, :], wcls_sb[0:128, 0, :],
                             start=True, stop=False)
            nc.tensor.matmul(cos_ps[:], clsT_sb[0:72, 1, :], wcls_sb[0:72, 1, :],
                             start=False, stop=True)
            t1_sb = sbuf.tile([RB, NM], F32, tag="t1")
            nc.vector.scalar_tensor_tensor(t1_sb[:], cos_ps[:], invx_sb[:],
                                           winvbS_sb[:], ALU.mult, ALU.mult)
            out_sb = sbuf.tile([RB, NM], F32, tag="out")
            nc.vector.tensor_tensor(out_sb[:], t1_sb[:], maskSM_sb[:], ALU.subtract)
            nc.sync.dma_start(Y[:], out_sb[:])

    nc.compile()
    return nc


def _prep_inputs(feat, label, mem_feat, wt, bt, wc, bc, w_sp, b_sp, w_cls):
    bf = ml_dtypes.bfloat16
    f32 = np.float32
    feat = np.ascontiguousarray(np.asarray(feat, dtype=f32))
    mem_feat = np.asarray(mem_feat, dtype=f32)
    wt = np.asarray(wt, dtype=f32)
    bt = np.asarray(bt, dtype=f32)
    wc = np.asarray(wc, dtype=f32)
    bc = np.asarray(bc, dtype=f32)
    w_sp = np.asarray(w_sp, dtype=f32)
    b_sp = np.asarray(b_sp, dtype=f32)
    w_cls = np.asarray(w_cls, dtype=f32)
    label = np.asarray(label)

    V = np.zeros((HW, 3), f32)
    V[:HW - 1, 0] = w_sp[0, 1:]
    V[:, 1] = w_sp[0, :]
    V[1:, 2] = w_sp[0, :HW - 1]
    vm = np.zeros((128, 2, 3), f32)
    vm[:, 0, :] = V[0:128]
    vm[0:68, 1, :] = V[128:HW]
    vm = vm.astype(bf)

    # mem_feat.T [2048,200] -> [128, 16, 200]
    mft = np.ascontiguousarray(
        mem_feat.T.reshape(NIT, 128, NM).transpose(1, 0, 2)).astype(bf)

    wclsT = np.zeros((128, 2, NM), f32)
    wclsT[:, 0, :] = w_cls.T[0:128]
    wclsT[0:72, 1, :] = w_cls.T[128:NM]

    bsp = np.full((BS, 1), b_sp[0] / N_CORES, f32)
    lbl_full = label.astype(f32).reshape(BS, 1)
    iota = np.broadcast_to(np.arange(NM, dtype=f32), (RB, NM)).copy()

    fv = feat.reshape(BS, C, HW)
    in_maps = []
    for c in range(N_CORES):
        J = slice(c * SH, (c + 1) * SH)
        # wt_sb[p, t, jc, it, j'] = wt[it*128+p, c*256+jc*128+j', t]
        wt_c = np.ascontiguousarray(
            wt[:, J, :].reshape(NIT, 128, 2, 128, 3)
            .transpose(1, 4, 2, 0, 3)).astype(bf)
        # wcT_sb[p, it, dt, o'] = wc[c*256+o', it*128+p, dt]
        wct_c = np.ascontiguousarray(
            wc[J].transpose(1, 2, 0).reshape(N_CORES, 2, 128, 3, SH)
            .transpose(2, 1, 0, 3, 4)).astype(bf)
        in_maps.append({
            "mft": mft,
            "wtc": wt_c,
            "wct": wct_c,
            "btbc": np.ascontiguousarray(
                np.stack([bt[J].reshape(2, 128).T, bc[J].reshape(2, 128).T],
                         axis=-1)),
            "ftc": np.ascontiguousarray(
                fv[:, J, :].transpose(2, 0, 1).reshape(HW, BS * SH)).astype(bf),
            "vm": vm, "wclsT": wclsT,
            "bsp": bsp, "lbl": lbl_full[c * RB:(c + 1) * RB], "iota": iota,
        })
    return in_maps


def kernel(**inputs) -> np.ndarray:
    global LAST_RESULT
    if "nc" not in _CACHE:
        _CACHE["nc"] = build_nc()
    nc = _CACHE["nc"]
    in_maps = _prep_inputs(**inputs)
    try:
        res = bass_utils.run_bass_kernel_spmd(
            nc, in_maps, core_ids=list(range(N_CORES)),
            trace=TRACE, **TRACE_KW,
        )
    except Exception:
        # transient NRT/device hiccups recover on retry
        res = bass_utils.run_bass_kernel_spmd(
            nc, in_maps, core_ids=list(range(N_CORES)),
            trace=TRACE, **TRACE_KW,
        )
    LAST_RESULT = res
    return np.concatenate(
        [np.asarray(res.results[c]["y"], dtype=np.float32) for c in range(N_CORES)],
        axis=0,
    )
